# revision 19
# baseline (speedup 1.0000x reference)
"""Dense graph-attention layer (GAT) on 8 Trainium2 NeuronCores.

Reference computation (all f32):
    h = x @ W                      # [N, F_OUT]
    f_src = h @ a_src              # [N]
    f_dst = h @ a_dst              # [N]
    e[i,j] = leaky_relu(f_src[i] + f_dst[j], 0.2), masked to -inf where adj==0
    alpha = softmax(e, axis=1)
    out = alpha @ h                # [N, F_OUT]

Sharding: output rows i are sharded across 8 cores (1024 rows each). Each
core receives its slice of adj transposed to [N, 1024] as a bf16 0/1 mask,
so j lands on partitions when tiled — the orientation the alpha @ h
contraction needs.

Key reformulation (exact, not approximate): with softmax's invariance to a
per-row scale, exp(f_src[i]) factors out of both the numerator and the
denominator and cancels. Writing B = exp(f_dst), D = exp(0.2 f_dst),
c = exp(-0.8 f_src):
    exp(leaky_relu(e)) = max(exp(e), exp(0.2 e))          (exp is monotone)
                       = exp(f_src) * max(B[j], c[i] * D[j])
so alpha rows can be computed from s[j,i] = mask[j,i] * max(B[j], c[i]*D[j])
directly. No per-element exp/activation is needed — only a dual-op
tensor_scalar (mult+max against two per-partition vectors) and a mask
multiply, both on VectorE in bf16. PE accumulates outT += h_tile.T @ s and
denom += ones.T @ s across all 64 j-tiles in PSUM. exp() runs only on tiny
f_src/f_dst vectors. f_src/f_dst are computed as x @ (W @ a) with the
weight-only products W @ a_src / W @ a_dst folded on the host, and the
per-j-block h tiles are built inside the main loop so the x @ W matmuls
overlap the mask DMA stream.
"""

import numpy as np
import ml_dtypes
from contextlib import ExitStack

import concourse.bacc as bacc
import concourse.tile as tile
from concourse import mybir
from concourse.bass_utils import run_bass_kernel_spmd

F32 = mybir.dt.float32
BF16 = mybir.dt.bfloat16
AF = mybir.ActivationFunctionType
OP = mybir.AluOpType

N = 8192
F_IN = 256
F_OUT = 128
N_CORES = 8
ROWS = N // N_CORES          # 1024 output rows per core
P = 128                      # partitions
JT = N // P                  # 64 j-tiles per core
IT = ROWS // P               # 8 i-tiles per core
SLOPE = 0.2

# Per-tile compute-form schedule (see main loop): D=VectorE-only,
# A=ScalarE relu + fused VectorE op, G=VectorE TS + GpSimd mask-mult.
import os
FORMS = os.environ.get("KFORMS", "DAGDDAGD")

LAST_EXEC_TIME_NS = None
LAST_RESULT = None


def _build_program():
    nc = bacc.Bacc("TRN2", target_bir_lowering=False, debug=False,
                   num_devices=N_CORES)

    mask = nc.dram_tensor("mask", [N, ROWS], BF16, kind="ExternalInput")
    xT = nc.dram_tensor("xT", [F_IN, N], BF16, kind="ExternalInput")
    xoT = nc.dram_tensor("xoT", [F_IN, ROWS], BF16, kind="ExternalInput")
    w_in = nc.dram_tensor("W", [F_IN, F_OUT], BF16, kind="ExternalInput")
    # wa_dst = W @ a_dst, wa_src = W @ a_src  (weight-only, host-folded)
    wad = nc.dram_tensor("wa_dst", [F_IN, 1], BF16, kind="ExternalInput")
    was = nc.dram_tensor("wa_src", [F_IN, 1], BF16, kind="ExternalInput")
    ident = nc.dram_tensor("ident", [P, P], F32, kind="ExternalInput")
    ones_rb = nc.dram_tensor("ones_rb", [1, P], BF16, kind="ExternalInput")
    ones_cb = nc.dram_tensor("ones_cb", [P, 1], BF16, kind="ExternalInput")
    out = nc.dram_tensor("out", [ROWS, F_OUT], F32, kind="ExternalOutput")

    with tile.TileContext(nc) as tc:
        with ExitStack() as ctx:
            persist = ctx.enter_context(tc.tile_pool(name="persist", bufs=1))
            opsum = ctx.enter_context(
                tc.tile_pool(name="opsum", bufs=1, space="PSUM"))

            xt_sb = persist.tile([P, 2 * N], BF16)     # xT k-halves
            c_bcast = persist.tile([P, ROWS], BF16)    # exp(-0.8 f_src) bcast
            b_col = persist.tile([P, JT], F32)         # exp(f_dst)
            d_col = persist.tile([P, JT], F32)         # exp(0.2 f_dst)
            id_sb = persist.tile([P, P], F32)
            ones_r_sb = persist.tile([1, P], BF16)
            ones_c_sb = persist.tile([P, 1], BF16)
            FA = F_OUT + 1                             # W k-half + wa_dst col
            w_sb = persist.tile([P, 2 * FA], BF16)     # [W_k | wa_dst_k] x2
            was_sb = persist.tile([P, 2], BF16)        # wa_src k-halves
            inv_col = persist.tile([P, IT], F32)

            xo_sb = persist.tile([P, 2 * ROWS], BF16)
            nc.sync.dma_start(xo_sb[:, 0:ROWS], xoT[0:P, :])
            nc.sync.dma_start(xo_sb[:, ROWS:2 * ROWS], xoT[P:2 * P, :])
            nc.sync.dma_start(id_sb[:], ident[:, :])
            nc.sync.dma_start(ones_r_sb[:], ones_rb[:, :])
            nc.sync.dma_start(ones_c_sb[:], ones_cb[:, :])
            nc.sync.dma_start(w_sb[:, 0:F_OUT], w_in[0:P, :])
            nc.sync.dma_start(w_sb[:, F_OUT:FA], wad[0:P, :])
            nc.sync.dma_start(w_sb[:, FA:FA + F_OUT], w_in[P:2 * P, :])
            nc.sync.dma_start(w_sb[:, FA + F_OUT:2 * FA], wad[P:2 * P, :])
            nc.sync.dma_start(was_sb[:, 0:1], was[0:P, :])
            nc.sync.dma_start(was_sb[:, 1:2], was[P:2 * P, :])
            # xT chunks are DMA'd inside the main loop, interleaved with the
            # mask stream, so early h-block matmuls start as soon as their
            # chunk lands and masks don't queue behind the whole xT.

            # ------------ prep: f_dst, f_src -> B, D, c vectors ---------
            with ExitStack() as pctx:
                prep = pctx.enter_context(tc.tile_pool(name="prep", bufs=1))
                ppsum = pctx.enter_context(
                    tc.tile_pool(name="ppsum", bufs=2, space="PSUM"))

                # f_src row = wa_src.T @ xoT -> c = exp(-0.8 f_src), bcast
                c_row = prep.tile([1, ROWS], BF16)
                for q in range(ROWS // 512):
                    pfs = ppsum.tile([1, 512], F32, tag="pp")
                    nc.tensor.matmul(
                        pfs[:], lhsT=was_sb[:, 0:1],
                        rhs=xo_sb[:, q * 512:(q + 1) * 512],
                        start=True, stop=False)
                    nc.tensor.matmul(
                        pfs[:], lhsT=was_sb[:, 1:2],
                        rhs=xo_sb[:, ROWS + q * 512:ROWS + (q + 1) * 512],
                        start=False, stop=True)
                    nc.scalar.activation(c_row[:, q * 512:(q + 1) * 512],
                                         pfs[:], AF.Exp, scale=-0.8)
                for q in range(ROWS // 512):
                    pcb = ppsum.tile([P, 512], F32, tag="pp")
                    nc.tensor.matmul(
                        pcb[:], lhsT=ones_r_sb[:],
                        rhs=c_row[:, q * 512:(q + 1) * 512],
                        start=True, stop=True)
                    nc.scalar.copy(c_bcast[:, q * 512:(q + 1) * 512], pcb[:])

            # ---------------- main loop over 64 j-tiles ----------------
            # Per-tile compute form: 'D' = dual-op TS + TT on VectorE,
            # 'A' = relu on ScalarE + fused scalar_tensor_tensor on VectorE,
            # 'G' = dual-op TS on VectorE + mask-mult on GpSimd.
            # Mixing spreads the elementwise work across three engines.
            fdcol_sb = persist.tile([P, JT], F32)
            negb_col = persist.tile([P, JT], F32)
            with ExitStack() as mctx:
                msk_pool = mctx.enter_context(tc.tile_pool(name="msk", bufs=6))
                m_pool = mctx.enter_context(tc.tile_pool(name="m", bufs=8))
                s_pool = mctx.enter_context(tc.tile_pool(name="s", bufs=8))
                h_pool = mctx.enter_context(tc.tile_pool(name="h", bufs=6))
                hpsum = mctx.enter_context(
                    tc.tile_pool(name="hpsum", bufs=4, space="PSUM"))

                psum_out = opsum.tile([P, ROWS], F32)   # outT accumulator
                psum_den = opsum.tile([1, ROWS], F32)   # denom accumulator

                CH = N // 8
                B4 = JT // 4
                hbs = [None] * JT
                mks = [None] * JT
                for q in range(B4):
                    # interleaved input streams: xT chunk + two mask pairs
                    if q % 2 == 0:
                        ch = q // 2
                        nc.sync.dma_start(xt_sb[:, ch * CH:(ch + 1) * CH],
                                          xT[0:P, ch * CH:(ch + 1) * CH])
                        nc.sync.dma_start(
                            xt_sb[:, N + ch * CH:N + (ch + 1) * CH],
                            xT[P:2 * P, ch * CH:(ch + 1) * CH])
                    for u in (0, 2):
                        jt = 4 * q + u
                        mk2 = msk_pool.tile([P, 2 * ROWS], BF16, tag="mk")
                        nc.sync.dma_start(
                            mk2[:].rearrange("p (two i) -> p two i", two=2),
                            mask[jt * P:(jt + 2) * P, :].rearrange(
                                "(two p) i -> p two i", two=2))
                        mks[jt] = mk2[:, 0:ROWS]
                        mks[jt + 1] = mk2[:, ROWS:2 * ROWS]

                    # h blocks + f_dst columns for 4 tiles, then batched exps
                    for u in range(4):
                        jt = 4 * q + u
                        hp = hpsum.tile([P, FA], F32, tag="hp")
                        nc.tensor.matmul(
                            hp[:], lhsT=xt_sb[:, jt * P:(jt + 1) * P],
                            rhs=w_sb[:, 0:FA], start=True, stop=False)
                        nc.tensor.matmul(
                            hp[:], lhsT=xt_sb[:, N + jt * P:N + (jt + 1) * P],
                            rhs=w_sb[:, FA:2 * FA], start=False, stop=True)
                        hb = h_pool.tile([P, P], BF16, tag="hb")
                        nc.scalar.copy(hb[:], hp[:, 0:F_OUT])
                        nc.scalar.copy(fdcol_sb[:, jt:jt + 1], hp[:, F_OUT:FA])
                        hbs[jt] = hb
                    q4 = slice(4 * q, 4 * q + 4)
                    nc.scalar.activation(b_col[:, q4], fdcol_sb[:, q4], AF.Exp)
                    nc.scalar.activation(d_col[:, q4], fdcol_sb[:, q4],
                                         AF.Exp, scale=SLOPE)
                    nc.scalar.mul(negb_col[:, q4], b_col[:, q4], -1.0)

                    for u in range(4):
                        jt = 4 * q + u
                        form = FORMS[jt % len(FORMS)]
                        s = s_pool.tile([P, ROWS], BF16, tag="s")
                        if form == "A":
                            r = m_pool.tile([P, ROWS], BF16, tag="m")
                            nc.scalar.activation(
                                r[:], c_bcast[:], AF.Relu,
                                bias=negb_col[:, jt:jt + 1],
                                scale=d_col[:, jt:jt + 1])
                            nc.vector.scalar_tensor_tensor(
                                s[:], r[:], b_col[:, jt:jt + 1], mks[jt],
                                op0=OP.add, op1=OP.mult)
                        else:
                            m = m_pool.tile([P, ROWS], BF16, tag="m")
                            nc.vector.tensor_scalar(
                                m[:], c_bcast[:], d_col[:, jt:jt + 1],
                                b_col[:, jt:jt + 1], op0=OP.mult, op1=OP.max)
                            if form == "G":
                                nc.gpsimd.tensor_tensor(
                                    s[:], m[:], mks[jt], op=OP.mult)
                            else:
                                nc.vector.tensor_tensor(
                                    s[:], m[:], mks[jt], op=OP.mult)

                        for hh in range(2):
                            sl = slice(hh * 512, (hh + 1) * 512)
                            nc.tensor.matmul(
                                psum_out[:, sl], lhsT=hbs[jt][:],
                                rhs=s[:, sl],
                                start=(jt == 0), stop=(jt == JT - 1))
                            nc.tensor.matmul(
                                psum_den[:, sl], lhsT=ones_c_sb[:],
                                rhs=s[:, sl],
                                start=(jt == 0), stop=(jt == JT - 1))

            # ---------------- epilogue: normalize + transpose ----------
            with ExitStack() as ectx:
                epi = ectx.enter_context(tc.tile_pool(name="epi", bufs=2))
                epsum = ectx.enter_context(
                    tc.tile_pool(name="epsum", bufs=2, space="PSUM"))

                den_row = epi.tile([1, ROWS], F32, tag="den")
                nc.scalar.copy(den_row[:], psum_den[:])
                den_col = epi.tile([P, IT], F32, tag="denc")
                for it in range(IT):
                    pdt = epsum.tile([P, 1], F32, tag="ep")
                    nc.tensor.transpose(
                        pdt[:], den_row[:, it * P:(it + 1) * P],
                        id_sb[0:1, 0:1])
                    nc.scalar.copy(den_col[:, it:it + 1], pdt[:])
                nc.vector.reciprocal(inv_col[:], den_col[:])

                outT_sb = epi.tile([P, ROWS], F32, tag="outT")
                nc.scalar.copy(outT_sb[:], psum_out[:])
                for it in range(IT):
                    ptr = epsum.tile([P, P], F32, tag="ep")
                    nc.tensor.transpose(
                        ptr[:], outT_sb[:, it * P:(it + 1) * P], id_sb[:])
                    ot = epi.tile([P, P], F32, tag="ot")
                    nc.vector.tensor_scalar_mul(
                        ot[:], ptr[:], inv_col[:, it:it + 1])
                    nc.sync.dma_start(out[it * P:(it + 1) * P, :], ot[:])

    nc.compile()
    return nc


_PROGRAM = None


def _get_program():
    global _PROGRAM
    if _PROGRAM is None:
        _PROGRAM = _build_program()
    return _PROGRAM


def kernel(x, adj, W, a_src, a_dst):
    global LAST_EXEC_TIME_NS, LAST_RESULT
    x = np.asarray(x, dtype=np.float32)
    adj = np.asarray(adj, dtype=np.float32)
    W = np.asarray(W, dtype=np.float32)
    a_src = np.asarray(a_src, dtype=np.float32).reshape(F_OUT)
    a_dst = np.asarray(a_dst, dtype=np.float32).reshape(F_OUT)

    nc = _get_program()

    bf = ml_dtypes.bfloat16
    xT = np.ascontiguousarray(x.T).astype(bf)
    in_common = {
        "xT": xT,
        "W": W.astype(bf),
        "wa_dst": (W @ a_dst).reshape(F_IN, 1).astype(bf),
        "wa_src": (W @ a_src).reshape(F_IN, 1).astype(bf),
        "ident": np.eye(P, dtype=np.float32),
        "ones_rb": np.ones((1, P), dtype=bf),
        "ones_cb": np.ones((P, 1), dtype=bf),
    }
    in_maps = []
    for c in range(N_CORES):
        rows = slice(c * ROWS, (c + 1) * ROWS)
        im = dict(in_common)
        im["mask"] = np.ascontiguousarray(adj[rows, :].T).astype(bf)
        im["xoT"] = np.ascontiguousarray(x[rows, :].T).astype(bf)
        in_maps.append(im)

    res = run_bass_kernel_spmd(nc, in_maps, core_ids=list(range(N_CORES)))
    LAST_EXEC_TIME_NS = res.exec_time_ns
    LAST_RESULT = res
    return np.concatenate(
        [res.results[c]["out"] for c in range(N_CORES)], axis=0)
